# revision 4
# baseline (speedup 1.0000x reference)
"""Trainium2 Bass kernel v2 for nn_KeypointLoss: data-parallel over batch (8 cores).

Changes vs v1 (109.2us):
 - preds shipped as fp8e4m3 (5.77MB instead of 11.5MB bf16): loss mean over
   720896 elements absorbs the rounding (~6e-4 rel err, gate is 2e-2).
   HBM stream drops 23.3MB -> 17.5MB (~50us at the measured 348 GB/s).
 - fully interleaved per-sample pipeline: gt(b) -> colmax(b)/cast(b) ->
   pred(b,s) -> sub -> square, so DVE/ACT/DMA all stream from t~7us
   (v1 ran two serial phases: 30us of DVE-only gt work, then 50us of
   ACT-bound squares).
 - colmax split DVE/gpsimd by keypoint (no merge needed), squares split
   ACT/DVE (TTR mult+accum), casts on ACT: every engine lands ~40-45us
   under the ~50us DMA stream.
 - consts packed into 2 tensors issued on the scalar (ACT) HWDGE ring so
   the sync ring starts the gt stream immediately (v1 spent 6us of ring
   time on 8 const DMAs first).
 - argmax phase II rowsum uses a max-reduce (exact first-occurrence even
   with duplicated row values) instead of sum.
"""
import sys
import numpy as np

sys.path.insert(0, "/opt/trn_rl_repo")

import ml_dtypes
import concourse.bacc as bacc
import concourse.mybir as mybir
import concourse.tile as tile
from concourse.bass import IndirectOffsetOnAxis
from concourse.bass_utils import run_bass_kernel_spmd

F32 = mybir.dt.float32
BF16 = mybir.dt.bfloat16
I32 = mybir.dt.int32
F8 = mybir.dt.float8e4

HM_NP = ml_dtypes.bfloat16

B_LOC = 4      # batch per core
S = 2          # stacks
K = 11         # keypoints
C = 7          # label channels
HW = 65536     # 256*256
P = 128        # partitions
FK = HW // P   # 512
NJ = B_LOC * K  # 44 (b,k) images per core
NSC = S * C     # 14 (s,c) pairs
KF = K * FK     # 5632 free elems per (b[,s]) tile

# --- tuning knobs ---
# Casts alternate ACT/DVE per b so each engine's per-sample work (~12.5us)
# matches the 12.4us/b DMA pace: DVE = colmax 5.9 + 2 subs 5.9 (+cast 2.9 on
# its turn); ACT = 2 squares 10 (+cast 5 on its turn). Squares are ACT-only
# (tensor_tensor_reduce crashes the device; gpsimd can't run TT/reduce-X).
CAST_ACT = [True, False, False, False]  # gt cast engine per b (ACT else DVE)

_CACHE = {}


def _consts():
    # c44 packs all [NJ, *] constants: negp 128 | negf 512 | sbase 2 |
    # rowbase 1 | blockind 4 | labels placeholder 14 (labels are per-run,
    # shipped separately)
    negp = np.broadcast_to((P - np.arange(P, dtype=np.float32))[None, :], (NJ, P))
    negf = np.broadcast_to((FK - np.arange(FK, dtype=np.float32))[None, :], (NJ, FK))
    b_of_j = np.arange(NJ) // K
    k_of_j = np.arange(NJ) % K
    sbase = ((b_of_j[:, None] * S + np.arange(S)[None, :]) * HW).astype(np.float32)
    rowbase = ((b_of_j * P + P) * K + k_of_j).astype(np.float32)[:, None]
    blockind = (b_of_j[:, None] == np.arange(B_LOC)[None, :]).astype(np.float32)
    c44 = np.concatenate([negp, negf, sbase, rowbase, blockind], axis=1)
    # c128 packs identity 128 | ones 1
    c128 = np.concatenate([np.eye(P, dtype=np.float32),
                           np.ones((P, 1), np.float32)], axis=1)
    return np.ascontiguousarray(c44), np.ascontiguousarray(c128)


def _build():
    nc = bacc.Bacc("TRN2", target_bir_lowering=False, debug=False,
                   enable_asserts=False, num_devices=8)
    hm = nc.dram_tensor("hm", [B_LOC, S, P, KF], BF16, kind="ExternalInput").ap()
    gt = nc.dram_tensor("gt", [B_LOC * P * K, FK], F32, kind="ExternalInput").ap()
    lb = nc.dram_tensor("lb", [B_LOC * S * HW, C], F32, kind="ExternalInput").ap()
    labels_bc = nc.dram_tensor("labels_bc", [NJ, NSC], F32, kind="ExternalInput").ap()
    c44_d = nc.dram_tensor("c44", [NJ, 647], F32, kind="ExternalInput").ap()
    c128_d = nc.dram_tensor("c128", [P, P + 1], F32, kind="ExternalInput").ap()
    hm_out = nc.dram_tensor("hm_out", [1, B_LOC * S], F32, kind="ExternalOutput").ap()
    lb_out = nc.dram_tensor("lb_out", [B_LOC, S], F32, kind="ExternalOutput").ap()

    # direct-load view of gt: [b, p, (k f)] with contiguous partition rows
    gt3v = gt.rearrange("(b p k) f -> b p (k f)", b=B_LOC, p=P)

    with tile.TileContext(nc) as tc:
        with (
            tc.tile_pool(name="gtp", bufs=2) as gtp,
            tc.tile_pool(name="gbf", bufs=2) as gbfp,
            tc.tile_pool(name="work", bufs=3) as work,
            tc.tile_pool(name="diffp", bufs=3) as diffp,
            tc.tile_pool(name="small", bufs=1) as small,
            tc.tile_pool(name="psum", bufs=2, space="PSUM") as psp,
        ):
            c44_t = small.tile([NJ, 647], F32, tag="c44")
            c128_t = small.tile([P, P + 1], F32, tag="c128")
            lab_t = small.tile([NJ, NSC], F32, tag="lab")
            negp_t = c44_t[:, 0:P]
            negf_t = c44_t[:, P:P + FK]
            sbase_t = c44_t[:, P + FK:P + FK + S]
            rowb_t = c44_t[:, P + FK + S:P + FK + S + 1]
            blk_t = c44_t[:, P + FK + S + 1:P + FK + S + 1 + B_LOC]
            id_t = c128_t[:, 0:P]
            ones_t = c128_t[:, P:P + 1]

            colmax = small.tile([P, NJ], F32, tag="colmax")
            acc = small.tile([P, B_LOC * S], F32, tag="acc")
            scr_s = small.tile([P, KF], BF16, tag="scr_s")
            scr_v = small.tile([P, KF], BF16, tag="scr_v")

            gt_ts = {}
            gbf_ts = {}

            # consts ride the scalar (ACT) HWDGE ring; the big streams all
            # share the sync ring in consumption order (dual-ring splitting
            # capped each ring at ~256GB/s and made the Tile scheduler
            # reorder the DVE queue - measured 128us vs 104us this way).
            nc.scalar.dma_start(out=c44_t[:], in_=c44_d)
            nc.scalar.dma_start(out=c128_t[:], in_=c128_d)
            nc.scalar.dma_start(out=lab_t[:], in_=labels_bc)

            KSPL = 6            # gt0 chunk boundary (keypoints)
            FSPL = KSPL * FK    # 3072

            def load_gt(b):
                gt_t = gtp.tile([P, KF], F32, tag="gt")
                if b <= 1:
                    # chunked so the colmax can start on the first half
                    # while the second is still in flight
                    nc.sync.dma_start(out=gt_t[:, 0:FSPL],
                                      in_=gt3v[b][:, 0:FSPL])
                    nc.sync.dma_start(out=gt_t[:, FSPL:KF],
                                      in_=gt3v[b][:, FSPL:KF])
                else:
                    nc.sync.dma_start(out=gt_t[:], in_=gt3v[b])
                gt_ts[b] = gt_t

            def load_pred(b, s, split=False):
                # plain HWDGE bf16: SWDGE cast-DMA shipping (fp8->bf16)
                # derates every engine clock ~1.2x for the whole run.
                pred_t = work.tile([P, KF], BF16, tag="pred")
                if split:
                    nc.sync.dma_start(out=pred_t[:, 0:KF // 2],
                                      in_=hm[b, s, :, 0:KF // 2])
                    nc.sync.dma_start(out=pred_t[:, KF // 2:KF],
                                      in_=hm[b, s, :, KF // 2:KF])
                else:
                    nc.sync.dma_start(out=pred_t[:], in_=hm[b, s])
                return pred_t

            def proc_gt(b):
                gt_t = gt_ts[b]
                gt3 = gt_t[:].rearrange("p (k f) -> p k f", k=K)
                gbf_t = gbfp.tile([P, KF], BF16, tag="gbf")
                if b == 0:
                    # chunked: compute starts as soon as the first 6
                    # keypoints land instead of the whole 2.88MB tile
                    nc.vector.tensor_reduce(
                        out=colmax[:, 0:KSPL], in_=gt3[:, 0:KSPL],
                        axis=mybir.AxisListType.X, op=mybir.AluOpType.max)
                    nc.scalar.copy(out=gbf_t[:, 0:FSPL], in_=gt_t[:, 0:FSPL])
                    nc.vector.tensor_reduce(
                        out=colmax[:, KSPL:K], in_=gt3[:, KSPL:K],
                        axis=mybir.AxisListType.X, op=mybir.AluOpType.max)
                    nc.scalar.copy(out=gbf_t[:, FSPL:KF], in_=gt_t[:, FSPL:KF])
                elif b == 1:
                    nc.vector.tensor_reduce(
                        out=colmax[:, K:K + KSPL], in_=gt3[:, 0:KSPL],
                        axis=mybir.AxisListType.X, op=mybir.AluOpType.max)
                    nc.vector.tensor_reduce(
                        out=colmax[:, K + KSPL:2 * K], in_=gt3[:, KSPL:K],
                        axis=mybir.AxisListType.X, op=mybir.AluOpType.max)
                    nc.vector.tensor_copy(out=gbf_t[:], in_=gt_t[:])
                else:
                    nc.vector.tensor_reduce(
                        out=colmax[:, b * K:(b + 1) * K], in_=gt3,
                        axis=mybir.AxisListType.X, op=mybir.AluOpType.max)
                    if CAST_ACT[b]:
                        nc.scalar.copy(out=gbf_t[:], in_=gt_t[:])
                    else:
                        nc.vector.tensor_copy(out=gbf_t[:], in_=gt_t[:])
                gbf_ts[b] = gbf_t

            def proc_pred(b, s, pred_t, sq_dve=False):
                diff_t = diffp.tile([P, KF], BF16, tag="diff")
                nc.vector.tensor_tensor(
                    out=diff_t[:], in0=pred_t[:], in1=gbf_ts[b][:],
                    op=mybir.AluOpType.subtract)
                col = b * S + s
                if sq_dve:
                    nc.vector.scalar_tensor_tensor(
                        out=scr_v[:], in0=diff_t[:], scalar=1.0,
                        in1=diff_t[:], op0=mybir.AluOpType.mult,
                        op1=mybir.AluOpType.mult,
                        accum_out=acc[:, col:col + 1])
                else:
                    nc.scalar.activation(
                        out=scr_s[:], in_=diff_t[:],
                        func=mybir.ActivationFunctionType.Square,
                        accum_out=acc[:, col:col + 1])

            def proc_pred_half(b, s, pred_t, half, accum):
                # sub on DVE, then square+accum ALSO on DVE via the stock
                # STT op (out=(d*1)*d, accum=sum): the stream-tail squares
                # would otherwise serialize on ACT behind the earlier ones.
                lo, hi = (0, KF // 2) if half == 0 else (KF // 2, KF)
                diff_t = diffp.tile([P, KF // 2], BF16, tag="diffh")
                nc.vector.tensor_tensor(
                    out=diff_t[:], in0=pred_t[:, lo:hi],
                    in1=gbf_ts[b][:, lo:hi], op=mybir.AluOpType.subtract)
                nc.scalar.activation(
                    out=scr_s[:, 0:KF // 2], in_=diff_t[:],
                    func=mybir.ActivationFunctionType.Square,
                    accum_out=accum)

            # ---- interleaved pipeline (v2b emission pattern) ----
            load_gt(0)
            p00 = load_pred(0, 0)
            p01 = load_pred(0, 1)
            load_gt(1)
            p10 = load_pred(1, 0)
            p11 = load_pred(1, 1)
            proc_gt(0)
            proc_pred(0, 0, p00)
            proc_pred(0, 1, p01)
            load_gt(2)
            p20 = load_pred(2, 0)
            p21 = load_pred(2, 1)
            proc_gt(1)
            proc_pred(1, 0, p10)
            proc_pred(1, 1, p11)
            load_gt(3)
            p30 = load_pred(3, 0)
            p31 = load_pred(3, 1, split=True)
            proc_gt(2)
            proc_pred(2, 0, p20)
            proc_pred(2, 1, p21)
            proc_gt(3)

            # ---- argmax stage (small ops; hides under the stream tail) ----
            cm_p = psp.tile([NJ, P], F32, tag="cmp", space="PSUM")
            nc.tensor.transpose(out=cm_p[:], in_=colmax[:], identity=id_t)
            cmT = small.tile([NJ, P], F32, tag="cmT")
            nc.vector.tensor_copy(out=cmT[:], in_=cm_p[:])
            gmax = small.tile([NJ, 1], F32, tag="gmax")
            nc.vector.tensor_reduce(out=gmax[:], in_=cmT[:],
                                    axis=mybir.AxisListType.X,
                                    op=mybir.AluOpType.max)
            maskT = small.tile([NJ, P], F32, tag="maskT")
            nc.vector.tensor_scalar(out=maskT[:], in0=cmT[:], scalar1=gmax[:],
                                    scalar2=None, op0=mybir.AluOpType.is_ge)
            scoreT = small.tile([NJ, P], F32, tag="scoreT")
            nc.vector.tensor_tensor(out=scoreT[:], in0=maskT[:], in1=negp_t,
                                    op=mybir.AluOpType.mult)
            pscore = small.tile([NJ, 1], F32, tag="pscore")
            nc.vector.tensor_reduce(out=pscore[:], in_=scoreT[:],
                                    axis=mybir.AxisListType.X,
                                    op=mybir.AluOpType.max)
            # row units: off_row = rowbase - pscore*K
            offr_f = small.tile([NJ, 1], F32, tag="offr_f")
            nc.vector.scalar_tensor_tensor(
                out=offr_f[:], in0=pscore[:], scalar=float(-K),
                in1=rowb_t, op0=mybir.AluOpType.mult,
                op1=mybir.AluOpType.add)
            offr_i = small.tile([NJ, 1], I32, tag="offr_i")
            nc.vector.tensor_copy(out=offr_i[:], in_=offr_f[:])
            rows = small.tile([NJ, FK], F32, tag="rows")
            nc.gpsimd.indirect_dma_start(
                out=rows[:], out_offset=None, in_=gt,
                in_offset=IndirectOffsetOnAxis(ap=offr_i[:], axis=0))

            # ---- b3 pred work, interleaved with the argmax chain; the
            # last (3,1) tile is processed in halves so the post-stream
            # serial tail is half a tile, with the two partial sums folded
            # by an accumulating PSUM matmul (PE is idle anyway) ----
            proc_pred(3, 0, p30)
            accA = small.tile([P, 1], F32, tag="accA")
            accB = small.tile([P, 1], F32, tag="accB")
            proc_pred_half(3, 1, p31, 0, accA[:])
            proc_pred_half(3, 1, p31, 1, accB[:])

            # rowscore = max((row >= gmax) * (512 - f)) = 512 - f*  (max, not
            # sum: exact first-occurrence even if the winning value repeats)
            rmsk = small.tile([NJ, FK], F32, tag="rmsk")
            nc.vector.scalar_tensor_tensor(
                out=rmsk[:], in0=rows[:], scalar=gmax[:],
                in1=negf_t, op0=mybir.AluOpType.is_ge,
                op1=mybir.AluOpType.mult)
            rs44 = small.tile([NJ, 1], F32, tag="rs44")
            nc.vector.tensor_reduce(out=rs44[:], in_=rmsk[:],
                                    axis=mybir.AxisListType.X,
                                    op=mybir.AluOpType.max)
            # flat = (128-pscore)*512 + (512-rs44)
            t1 = small.tile([NJ, 1], F32, tag="t1")
            nc.vector.tensor_scalar(out=t1[:], in0=pscore[:], scalar1=-512.0,
                                    scalar2=None, op0=mybir.AluOpType.mult)
            flatf = small.tile([NJ, 1], F32, tag="flatf")
            nc.vector.scalar_tensor_tensor(
                out=flatf[:], in0=t1[:], scalar=float(P * FK + FK),
                in1=rs44[:], op0=mybir.AluOpType.add,
                op1=mybir.AluOpType.subtract)
            # lb is host-transposed to [B*S*HW, C]: one offset per
            # (image, stack) fetches all C=7 channels contiguously.
            off2_f = small.tile([NJ, S], F32, tag="off2_f")
            nc.vector.tensor_scalar(out=off2_f[:], in0=sbase_t,
                                    scalar1=flatf[:], scalar2=None,
                                    op0=mybir.AluOpType.add)
            off2_i = small.tile([NJ, S], I32, tag="off2_i")
            nc.vector.tensor_copy(out=off2_i[:], in_=off2_f[:])
            gath = small.tile([NJ, NSC], F32, tag="gath")
            for s in range(S):
                nc.gpsimd.indirect_dma_start(
                    out=gath[:, s * C:(s + 1) * C], out_offset=None,
                    in_=lb,
                    in_offset=IndirectOffsetOnAxis(ap=off2_i[:, s:s + 1], axis=0))

            # ---- label-loss tail + outputs ----
            ldiff = small.tile([NJ, NSC], F32, tag="ldiff")
            nc.vector.tensor_tensor(out=ldiff[:], in0=gath[:], in1=lab_t[:],
                                    op=mybir.AluOpType.subtract)
            lsq = small.tile([NJ, NSC], F32, tag="lsq")
            nc.scalar.activation(out=lsq[:], in_=ldiff[:],
                                 func=mybir.ActivationFunctionType.Square)
            persum = small.tile([NJ, S], F32, tag="persum")
            nc.vector.tensor_reduce(
                out=persum[:],
                in_=lsq[:].rearrange("j (s c) -> j s c", s=S),
                axis=mybir.AxisListType.X, op=mybir.AluOpType.add)
            lb_p = psp.tile([B_LOC, S], F32, tag="lbp", space="PSUM")
            nc.tensor.matmul(out=lb_p[:], lhsT=blk_t, rhs=persum[:],
                             start=True, stop=True)
            lb_s = small.tile([B_LOC, S], F32, tag="lbs")
            nc.scalar.activation(out=lb_s[:], in_=lb_p[:],
                                 func=mybir.ActivationFunctionType.Copy,
                                 scale=1.0 / (K * C))
            nc.sync.dma_start(out=lb_out, in_=lb_s[:])

            # col 7's sum lives in accA/accB (the (3,1) tile was processed
            # in halves); fold them with an accumulating PSUM matmul.
            hm_p = psp.tile([1, B_LOC * S], F32, tag="hmp", space="PSUM")
            nc.tensor.matmul(out=hm_p[:, 0:7], lhsT=ones_t, rhs=acc[:, 0:7],
                             start=True, stop=True)
            nc.tensor.matmul(out=hm_p[:, 7:8], lhsT=ones_t, rhs=accA[:],
                             start=True, stop=False)
            nc.tensor.matmul(out=hm_p[:, 7:8], lhsT=ones_t, rhs=accB[:],
                             start=False, stop=True)
            hm_s = small.tile([1, B_LOC * S], F32, tag="hms")
            nc.scalar.activation(out=hm_s[:], in_=hm_p[:],
                                 func=mybir.ActivationFunctionType.Copy,
                                 scale=1.0 / (K * HW))
            nc.sync.dma_start(out=hm_out, in_=hm_s[:])

    nc.compile()
    return nc


def _get_nc():
    if "nc" not in _CACHE:
        _CACHE["nc"] = _build()
    return _CACHE["nc"]


def make_in_maps(combined_hm_preds, combined_lb_preds, heatmaps, labels):
    c44, c128 = _consts()
    in_maps = []
    for c in range(8):
        sl = slice(c * B_LOC, (c + 1) * B_LOC)
        lab = np.asarray(labels[sl], np.float32)  # [4, 11, 7]
        lab_bc = np.broadcast_to(
            lab[:, :, None, :], (B_LOC, K, S, C)).reshape(NJ, NSC)
        # pre-transpose so partition rows are contiguous DMA lines
        hm_r = np.asarray(combined_hm_preds[sl], np.float32).reshape(
            B_LOC, S, K, P, FK).transpose(0, 1, 3, 2, 4).reshape(
            B_LOC, S, P, KF).astype(HM_NP)
        gt_r = np.asarray(heatmaps[sl], np.float32).reshape(
            B_LOC, K, P, FK).transpose(0, 2, 1, 3).reshape(B_LOC * P * K, FK)
        lb_r = np.asarray(combined_lb_preds[sl], np.float32).reshape(
            B_LOC, S, C, HW).transpose(0, 1, 3, 2).reshape(B_LOC * S * HW, C)
        m = {
            "hm": np.ascontiguousarray(hm_r),
            "gt": np.ascontiguousarray(gt_r),
            "lb": np.ascontiguousarray(lb_r),
            "labels_bc": np.ascontiguousarray(lab_bc),
            "c44": c44,
            "c128": c128,
        }
        in_maps.append(m)
    return in_maps


def run(in_maps, trace=False, **kw):
    nc = _get_nc()
    return run_bass_kernel_spmd(nc, in_maps, list(range(8)), trace=trace, **kw)


def kernel(combined_hm_preds, combined_lb_preds, heatmaps, labels):
    in_maps = make_in_maps(combined_hm_preds, combined_lb_preds, heatmaps,
                           labels)
    res = run(in_maps).results
    combined = np.concatenate(
        [r["hm_out"].reshape(B_LOC, S) for r in res], axis=0)
    labels_loss = np.concatenate([r["lb_out"] for r in res], axis=0)
    return combined.astype(np.float32), labels_loss.astype(np.float32)


# revision 5
# speedup vs baseline: 1.0281x; 1.0281x over previous
"""Trainium2 Bass kernel v2f for nn_KeypointLoss: data-parallel over batch
(8 cores). Measured 92.3us (v1 baseline: 109.2us); correctness 3.9e-05.

Per core (4 samples): gt heatmaps stream f32 (11.5MB, argmax must be
bit-exact) and preds stream bf16 (11.5MB), all on the single sync HWDGE
ring in consumption order (342-403 GB/s busy-rate, varies per run).
Fully interleaved per-sample pipeline: gt(b) -> colmax(b)+cast(b) ->
sub(b,s) -> square(b,s). DVE is the pacer (colmax 6.0us + 2 subs 3.1us
+ cast 3.1us per sample, near-gapless 14->76us); ACT does the squares
(5.0us each) plus only b0's cast (an ACT cast for any later sample sits
behind earlier squares in the ACT queue and stalls the subs ~5us, so
casts b1-b3 run on DVE). gt0/gt1 loads and colmaxes are chunked (6/5
keypoints) to start compute earlier; consts are packed into 2 tensors
on the scalar HWDGE ring; the last pred tile (3,1) is processed in
halves whose partial sums fold via an accumulating PSUM matmul.

Pitfalls measured on HW (do NOT revisit without remeasuring):
 - SWDGE cast-DMA shipping preds as fp8 derates EVERY engine clock
   ~1.2x for the whole run (115us); the same derate appears
   spontaneously on ~1 in 6 runs.
 - Splitting streams across both HWDGE rings caps each at ~256 GB/s
   and makes the Tile scheduler hoist all colmaxes before the subs.
 - tensor_tensor_reduce crashes the device; use scalar_tensor_tensor
   with accum_out for fused square+sum on DVE.
 - fp8 TensorTensor runs 1x (6.0us); DVE f32 reduce is 1 elem/cycle
   for any AP shape; f32->bf16 copy and bf16 TT run 2/cycle; ACT is
   1/cycle at 1.2GHz for all dtypes.

Argmax scheme (exact, first-occurrence tie-break like jnp.argmax):
 - colmax[p, j] = max_f gt[j][p, f]; PE-transpose -> [44,128]; global
   max -> winning partition p* (lowest tied p via max of mask*(128-p))
 - indirect-DMA gather of row (p*, j) for all 44 images -> [44, 512]
 - rowscore[j] = max_f (row >= gmax)*(512-f) = 512 - f* (max-reduce:
   exact first occurrence even if the winning value repeats)
 - flat = p**512 + f*; per-(image,stack) indirect-DMA gathers the C=7
   label-pred channels at the argmax locations ([B*S*HW, C] layout).
"""
import sys
import numpy as np

sys.path.insert(0, "/opt/trn_rl_repo")

import ml_dtypes
import concourse.bacc as bacc
import concourse.mybir as mybir
import concourse.tile as tile
from concourse.bass import IndirectOffsetOnAxis
from concourse.bass_utils import run_bass_kernel_spmd

F32 = mybir.dt.float32
BF16 = mybir.dt.bfloat16
I32 = mybir.dt.int32
F8 = mybir.dt.float8e4

HM_NP = ml_dtypes.bfloat16

B_LOC = 4      # batch per core
S = 2          # stacks
K = 11         # keypoints
C = 7          # label channels
HW = 65536     # 256*256
P = 128        # partitions
FK = HW // P   # 512
NJ = B_LOC * K  # 44 (b,k) images per core
NSC = S * C     # 14 (s,c) pairs
KF = K * FK     # 5632 free elems per (b[,s]) tile

# --- tuning knobs ---
# Casts alternate ACT/DVE per b so each engine's per-sample work (~12.5us)
# matches the 12.4us/b DMA pace: DVE = colmax 5.9 + 2 subs 5.9 (+cast 2.9 on
# its turn); ACT = 2 squares 10 (+cast 5 on its turn). Squares are ACT-only
# (tensor_tensor_reduce crashes the device; gpsimd can't run TT/reduce-X).
CAST_ACT = [True, False, False, False]  # gt cast engine per b (ACT else DVE)

_CACHE = {}


def _consts():
    # c44 packs all [NJ, *] constants: negp 128 | negf 512 | sbase 2 |
    # rowbase 1 | blockind 4 | labels placeholder 14 (labels are per-run,
    # shipped separately)
    negp = np.broadcast_to((P - np.arange(P, dtype=np.float32))[None, :], (NJ, P))
    negf = np.broadcast_to((FK - np.arange(FK, dtype=np.float32))[None, :], (NJ, FK))
    b_of_j = np.arange(NJ) // K
    k_of_j = np.arange(NJ) % K
    sbase = ((b_of_j[:, None] * S + np.arange(S)[None, :]) * HW).astype(np.float32)
    rowbase = ((b_of_j * P + P) * K + k_of_j).astype(np.float32)[:, None]
    blockind = (b_of_j[:, None] == np.arange(B_LOC)[None, :]).astype(np.float32)
    c44 = np.concatenate([negp, negf, sbase, rowbase, blockind], axis=1)
    # c128 packs identity 128 | ones 1
    c128 = np.concatenate([np.eye(P, dtype=np.float32),
                           np.ones((P, 1), np.float32)], axis=1)
    return np.ascontiguousarray(c44), np.ascontiguousarray(c128)


def _build():
    nc = bacc.Bacc("TRN2", target_bir_lowering=False, debug=False,
                   enable_asserts=False, num_devices=8)
    hm = nc.dram_tensor("hm", [B_LOC, S, P, KF], BF16, kind="ExternalInput").ap()
    gt = nc.dram_tensor("gt", [B_LOC * P * K, FK], F32, kind="ExternalInput").ap()
    lb = nc.dram_tensor("lb", [B_LOC * S * HW, C], F32, kind="ExternalInput").ap()
    labels_bc = nc.dram_tensor("labels_bc", [NJ, NSC], F32, kind="ExternalInput").ap()
    c44_d = nc.dram_tensor("c44", [NJ, 647], F32, kind="ExternalInput").ap()
    c128_d = nc.dram_tensor("c128", [P, P + 1], F32, kind="ExternalInput").ap()
    hm_out = nc.dram_tensor("hm_out", [1, B_LOC * S], F32, kind="ExternalOutput").ap()
    lb_out = nc.dram_tensor("lb_out", [B_LOC, S], F32, kind="ExternalOutput").ap()

    # direct-load view of gt: [b, p, (k f)] with contiguous partition rows
    gt3v = gt.rearrange("(b p k) f -> b p (k f)", b=B_LOC, p=P)

    with tile.TileContext(nc) as tc:
        with (
            tc.tile_pool(name="gtp", bufs=2) as gtp,
            tc.tile_pool(name="gbf", bufs=2) as gbfp,
            tc.tile_pool(name="work", bufs=3) as work,
            tc.tile_pool(name="diffp", bufs=3) as diffp,
            tc.tile_pool(name="small", bufs=1) as small,
            tc.tile_pool(name="psum", bufs=2, space="PSUM") as psp,
        ):
            c44_t = small.tile([NJ, 647], F32, tag="c44")
            c128_t = small.tile([P, P + 1], F32, tag="c128")
            lab_t = small.tile([NJ, NSC], F32, tag="lab")
            negp_t = c44_t[:, 0:P]
            negf_t = c44_t[:, P:P + FK]
            sbase_t = c44_t[:, P + FK:P + FK + S]
            rowb_t = c44_t[:, P + FK + S:P + FK + S + 1]
            blk_t = c44_t[:, P + FK + S + 1:P + FK + S + 1 + B_LOC]
            id_t = c128_t[:, 0:P]
            ones_t = c128_t[:, P:P + 1]

            colmax = small.tile([P, NJ], F32, tag="colmax")
            acc = small.tile([P, B_LOC * S], F32, tag="acc")
            scr_s = small.tile([P, KF], BF16, tag="scr_s")
            scr_v = small.tile([P, KF], BF16, tag="scr_v")

            gt_ts = {}
            gbf_ts = {}

            # consts ride the scalar (ACT) HWDGE ring; the big streams all
            # share the sync ring in consumption order (dual-ring splitting
            # capped each ring at ~256GB/s and made the Tile scheduler
            # reorder the DVE queue - measured 128us vs 104us this way).
            nc.scalar.dma_start(out=c44_t[:], in_=c44_d)
            nc.scalar.dma_start(out=c128_t[:], in_=c128_d)
            nc.scalar.dma_start(out=lab_t[:], in_=labels_bc)

            KSPL = 6            # gt0 chunk boundary (keypoints)
            FSPL = KSPL * FK    # 3072

            def load_gt(b):
                gt_t = gtp.tile([P, KF], F32, tag="gt")
                if b <= 1:
                    # chunked so the colmax can start on the first half
                    # while the second is still in flight
                    nc.sync.dma_start(out=gt_t[:, 0:FSPL],
                                      in_=gt3v[b][:, 0:FSPL])
                    nc.sync.dma_start(out=gt_t[:, FSPL:KF],
                                      in_=gt3v[b][:, FSPL:KF])
                else:
                    nc.sync.dma_start(out=gt_t[:], in_=gt3v[b])
                gt_ts[b] = gt_t

            def load_pred(b, s, split=False):
                # plain HWDGE bf16: SWDGE cast-DMA shipping (fp8->bf16)
                # derates every engine clock ~1.2x for the whole run.
                pred_t = work.tile([P, KF], BF16, tag="pred")
                if split:
                    nc.sync.dma_start(out=pred_t[:, 0:KF // 2],
                                      in_=hm[b, s, :, 0:KF // 2])
                    nc.sync.dma_start(out=pred_t[:, KF // 2:KF],
                                      in_=hm[b, s, :, KF // 2:KF])
                else:
                    nc.sync.dma_start(out=pred_t[:], in_=hm[b, s])
                return pred_t

            def proc_gt(b):
                gt_t = gt_ts[b]
                gt3 = gt_t[:].rearrange("p (k f) -> p k f", k=K)
                gbf_t = gbfp.tile([P, KF], BF16, tag="gbf")
                if b == 0:
                    # chunked: compute starts as soon as the first 6
                    # keypoints land instead of the whole 2.88MB tile
                    nc.vector.tensor_reduce(
                        out=colmax[:, 0:KSPL], in_=gt3[:, 0:KSPL],
                        axis=mybir.AxisListType.X, op=mybir.AluOpType.max)
                    nc.scalar.copy(out=gbf_t[:, 0:FSPL], in_=gt_t[:, 0:FSPL])
                    nc.vector.tensor_reduce(
                        out=colmax[:, KSPL:K], in_=gt3[:, KSPL:K],
                        axis=mybir.AxisListType.X, op=mybir.AluOpType.max)
                    nc.scalar.copy(out=gbf_t[:, FSPL:KF], in_=gt_t[:, FSPL:KF])
                elif b == 1:
                    nc.vector.tensor_reduce(
                        out=colmax[:, K:K + KSPL], in_=gt3[:, 0:KSPL],
                        axis=mybir.AxisListType.X, op=mybir.AluOpType.max)
                    nc.vector.tensor_reduce(
                        out=colmax[:, K + KSPL:2 * K], in_=gt3[:, KSPL:K],
                        axis=mybir.AxisListType.X, op=mybir.AluOpType.max)
                    nc.vector.tensor_copy(out=gbf_t[:], in_=gt_t[:])
                else:
                    nc.vector.tensor_reduce(
                        out=colmax[:, b * K:(b + 1) * K], in_=gt3,
                        axis=mybir.AxisListType.X, op=mybir.AluOpType.max)
                    if CAST_ACT[b]:
                        nc.scalar.copy(out=gbf_t[:], in_=gt_t[:])
                    else:
                        nc.vector.tensor_copy(out=gbf_t[:], in_=gt_t[:])
                gbf_ts[b] = gbf_t

            def proc_pred(b, s, pred_t, sq_dve=False):
                diff_t = diffp.tile([P, KF], BF16, tag="diff")
                nc.vector.tensor_tensor(
                    out=diff_t[:], in0=pred_t[:], in1=gbf_ts[b][:],
                    op=mybir.AluOpType.subtract)
                col = b * S + s
                if sq_dve:
                    nc.vector.scalar_tensor_tensor(
                        out=scr_v[:], in0=diff_t[:], scalar=1.0,
                        in1=diff_t[:], op0=mybir.AluOpType.mult,
                        op1=mybir.AluOpType.mult,
                        accum_out=acc[:, col:col + 1])
                else:
                    nc.scalar.activation(
                        out=scr_s[:], in_=diff_t[:],
                        func=mybir.ActivationFunctionType.Square,
                        accum_out=acc[:, col:col + 1])

            def proc_pred_half(b, s, pred_t, half, accum):
                # sub on DVE, then square+accum ALSO on DVE via the stock
                # STT op (out=(d*1)*d, accum=sum): the stream-tail squares
                # would otherwise serialize on ACT behind the earlier ones.
                lo, hi = (0, KF // 2) if half == 0 else (KF // 2, KF)
                diff_t = diffp.tile([P, KF // 2], BF16, tag="diffh")
                nc.vector.tensor_tensor(
                    out=diff_t[:], in0=pred_t[:, lo:hi],
                    in1=gbf_ts[b][:, lo:hi], op=mybir.AluOpType.subtract)
                nc.scalar.activation(
                    out=scr_s[:, 0:KF // 2], in_=diff_t[:],
                    func=mybir.ActivationFunctionType.Square,
                    accum_out=accum)

            # ---- interleaved pipeline (v2b emission pattern) ----
            load_gt(0)
            p00 = load_pred(0, 0)
            p01 = load_pred(0, 1)
            load_gt(1)
            p10 = load_pred(1, 0)
            p11 = load_pred(1, 1)
            proc_gt(0)
            proc_pred(0, 0, p00)
            proc_pred(0, 1, p01)
            load_gt(2)
            p20 = load_pred(2, 0)
            p21 = load_pred(2, 1)
            proc_gt(1)
            proc_pred(1, 0, p10)
            proc_pred(1, 1, p11)
            load_gt(3)
            p30 = load_pred(3, 0)
            p31 = load_pred(3, 1, split=True)
            proc_gt(2)
            proc_pred(2, 0, p20)
            proc_pred(2, 1, p21)
            proc_gt(3)

            # ---- argmax stage (small ops; hides under the stream tail) ----
            cm_p = psp.tile([NJ, P], F32, tag="cmp", space="PSUM")
            nc.tensor.transpose(out=cm_p[:], in_=colmax[:], identity=id_t)
            cmT = small.tile([NJ, P], F32, tag="cmT")
            nc.vector.tensor_copy(out=cmT[:], in_=cm_p[:])
            gmax = small.tile([NJ, 1], F32, tag="gmax")
            nc.vector.tensor_reduce(out=gmax[:], in_=cmT[:],
                                    axis=mybir.AxisListType.X,
                                    op=mybir.AluOpType.max)
            maskT = small.tile([NJ, P], F32, tag="maskT")
            nc.vector.tensor_scalar(out=maskT[:], in0=cmT[:], scalar1=gmax[:],
                                    scalar2=None, op0=mybir.AluOpType.is_ge)
            scoreT = small.tile([NJ, P], F32, tag="scoreT")
            nc.vector.tensor_tensor(out=scoreT[:], in0=maskT[:], in1=negp_t,
                                    op=mybir.AluOpType.mult)
            pscore = small.tile([NJ, 1], F32, tag="pscore")
            nc.vector.tensor_reduce(out=pscore[:], in_=scoreT[:],
                                    axis=mybir.AxisListType.X,
                                    op=mybir.AluOpType.max)
            # row units: off_row = rowbase - pscore*K
            offr_f = small.tile([NJ, 1], F32, tag="offr_f")
            nc.vector.scalar_tensor_tensor(
                out=offr_f[:], in0=pscore[:], scalar=float(-K),
                in1=rowb_t, op0=mybir.AluOpType.mult,
                op1=mybir.AluOpType.add)
            offr_i = small.tile([NJ, 1], I32, tag="offr_i")
            nc.vector.tensor_copy(out=offr_i[:], in_=offr_f[:])
            rows = small.tile([NJ, FK], F32, tag="rows")
            nc.gpsimd.indirect_dma_start(
                out=rows[:], out_offset=None, in_=gt,
                in_offset=IndirectOffsetOnAxis(ap=offr_i[:], axis=0))

            # ---- b3 pred work, interleaved with the argmax chain; the
            # last (3,1) tile is processed in halves so the post-stream
            # serial tail is half a tile, with the two partial sums folded
            # by an accumulating PSUM matmul (PE is idle anyway) ----
            proc_pred(3, 0, p30)
            accA = small.tile([P, 1], F32, tag="accA")
            accB = small.tile([P, 1], F32, tag="accB")
            proc_pred_half(3, 1, p31, 0, accA[:])
            proc_pred_half(3, 1, p31, 1, accB[:])

            # rowscore = max((row >= gmax) * (512 - f)) = 512 - f*  (max, not
            # sum: exact first-occurrence even if the winning value repeats)
            rmsk = small.tile([NJ, FK], F32, tag="rmsk")
            nc.vector.scalar_tensor_tensor(
                out=rmsk[:], in0=rows[:], scalar=gmax[:],
                in1=negf_t, op0=mybir.AluOpType.is_ge,
                op1=mybir.AluOpType.mult)
            rs44 = small.tile([NJ, 1], F32, tag="rs44")
            nc.vector.tensor_reduce(out=rs44[:], in_=rmsk[:],
                                    axis=mybir.AxisListType.X,
                                    op=mybir.AluOpType.max)
            # flat = (128-pscore)*512 + (512-rs44)
            t1 = small.tile([NJ, 1], F32, tag="t1")
            nc.vector.tensor_scalar(out=t1[:], in0=pscore[:], scalar1=-512.0,
                                    scalar2=None, op0=mybir.AluOpType.mult)
            flatf = small.tile([NJ, 1], F32, tag="flatf")
            nc.vector.scalar_tensor_tensor(
                out=flatf[:], in0=t1[:], scalar=float(P * FK + FK),
                in1=rs44[:], op0=mybir.AluOpType.add,
                op1=mybir.AluOpType.subtract)
            # lb is host-transposed to [B*S*HW, C]: one offset per
            # (image, stack) fetches all C=7 channels contiguously.
            off2_f = small.tile([NJ, S], F32, tag="off2_f")
            nc.vector.tensor_scalar(out=off2_f[:], in0=sbase_t,
                                    scalar1=flatf[:], scalar2=None,
                                    op0=mybir.AluOpType.add)
            off2_i = small.tile([NJ, S], I32, tag="off2_i")
            nc.vector.tensor_copy(out=off2_i[:], in_=off2_f[:])
            gath = small.tile([NJ, NSC], F32, tag="gath")
            for s in range(S):
                nc.gpsimd.indirect_dma_start(
                    out=gath[:, s * C:(s + 1) * C], out_offset=None,
                    in_=lb,
                    in_offset=IndirectOffsetOnAxis(ap=off2_i[:, s:s + 1], axis=0))

            # ---- label-loss tail + outputs ----
            ldiff = small.tile([NJ, NSC], F32, tag="ldiff")
            nc.vector.tensor_tensor(out=ldiff[:], in0=gath[:], in1=lab_t[:],
                                    op=mybir.AluOpType.subtract)
            lsq = small.tile([NJ, NSC], F32, tag="lsq")
            nc.scalar.activation(out=lsq[:], in_=ldiff[:],
                                 func=mybir.ActivationFunctionType.Square)
            persum = small.tile([NJ, S], F32, tag="persum")
            nc.vector.tensor_reduce(
                out=persum[:],
                in_=lsq[:].rearrange("j (s c) -> j s c", s=S),
                axis=mybir.AxisListType.X, op=mybir.AluOpType.add)
            lb_p = psp.tile([B_LOC, S], F32, tag="lbp", space="PSUM")
            nc.tensor.matmul(out=lb_p[:], lhsT=blk_t, rhs=persum[:],
                             start=True, stop=True)
            lb_s = small.tile([B_LOC, S], F32, tag="lbs")
            nc.scalar.activation(out=lb_s[:], in_=lb_p[:],
                                 func=mybir.ActivationFunctionType.Copy,
                                 scale=1.0 / (K * C))
            nc.sync.dma_start(out=lb_out, in_=lb_s[:])

            # col 7's sum lives in accA/accB (the (3,1) tile was processed
            # in halves); fold them with an accumulating PSUM matmul.
            hm_p = psp.tile([1, B_LOC * S], F32, tag="hmp", space="PSUM")
            nc.tensor.matmul(out=hm_p[:, 0:7], lhsT=ones_t, rhs=acc[:, 0:7],
                             start=True, stop=True)
            nc.tensor.matmul(out=hm_p[:, 7:8], lhsT=ones_t, rhs=accA[:],
                             start=True, stop=False)
            nc.tensor.matmul(out=hm_p[:, 7:8], lhsT=ones_t, rhs=accB[:],
                             start=False, stop=True)
            hm_s = small.tile([1, B_LOC * S], F32, tag="hms")
            nc.scalar.activation(out=hm_s[:], in_=hm_p[:],
                                 func=mybir.ActivationFunctionType.Copy,
                                 scale=1.0 / (K * HW))
            nc.sync.dma_start(out=hm_out, in_=hm_s[:])

    nc.compile()
    return nc


def _get_nc():
    if "nc" not in _CACHE:
        _CACHE["nc"] = _build()
    return _CACHE["nc"]


def make_in_maps(combined_hm_preds, combined_lb_preds, heatmaps, labels):
    c44, c128 = _consts()
    in_maps = []
    for c in range(8):
        sl = slice(c * B_LOC, (c + 1) * B_LOC)
        lab = np.asarray(labels[sl], np.float32)  # [4, 11, 7]
        lab_bc = np.broadcast_to(
            lab[:, :, None, :], (B_LOC, K, S, C)).reshape(NJ, NSC)
        # pre-transpose so partition rows are contiguous DMA lines
        hm_r = np.asarray(combined_hm_preds[sl], np.float32).reshape(
            B_LOC, S, K, P, FK).transpose(0, 1, 3, 2, 4).reshape(
            B_LOC, S, P, KF).astype(HM_NP)
        gt_r = np.asarray(heatmaps[sl], np.float32).reshape(
            B_LOC, K, P, FK).transpose(0, 2, 1, 3).reshape(B_LOC * P * K, FK)
        lb_r = np.asarray(combined_lb_preds[sl], np.float32).reshape(
            B_LOC, S, C, HW).transpose(0, 1, 3, 2).reshape(B_LOC * S * HW, C)
        m = {
            "hm": np.ascontiguousarray(hm_r),
            "gt": np.ascontiguousarray(gt_r),
            "lb": np.ascontiguousarray(lb_r),
            "labels_bc": np.ascontiguousarray(lab_bc),
            "c44": c44,
            "c128": c128,
        }
        in_maps.append(m)
    return in_maps


def run(in_maps, trace=False, **kw):
    nc = _get_nc()
    return run_bass_kernel_spmd(nc, in_maps, list(range(8)), trace=trace, **kw)


def kernel(combined_hm_preds, combined_lb_preds, heatmaps, labels):
    in_maps = make_in_maps(combined_hm_preds, combined_lb_preds, heatmaps,
                           labels)
    res = run(in_maps).results
    combined = np.concatenate(
        [r["hm_out"].reshape(B_LOC, S) for r in res], axis=0)
    labels_loss = np.concatenate([r["lb_out"] for r in res], axis=0)
    return combined.astype(np.float32), labels_loss.astype(np.float32)


# revision 7
# speedup vs baseline: 1.0945x; 1.0646x over previous
"""Trainium2 Bass kernel v2j for nn_KeypointLoss: data-parallel over batch
(8 cores). Measured 92.3us fast-DMA / 97.5us slow-DMA (v1 baseline:
109.2us); correctness 3.9e-05. v2j = v2f + gt3 loaded right behind gt2
on the ring so its colmax/cast run mid-stream on slow-DMA runs instead
of serially after the last byte (v2f slow-run samples were ~100.3us).

Per core (4 samples): gt heatmaps stream f32 (11.5MB, argmax must be
bit-exact) and preds stream bf16 (11.5MB), all on the single sync HWDGE
ring in consumption order (342-403 GB/s busy-rate, varies per run).
Fully interleaved per-sample pipeline: gt(b) -> colmax(b)+cast(b) ->
sub(b,s) -> square(b,s). DVE is the pacer (colmax 6.0us + 2 subs 3.1us
+ cast 3.1us per sample, near-gapless 14->76us); ACT does the squares
(5.0us each) plus only b0's cast (an ACT cast for any later sample sits
behind earlier squares in the ACT queue and stalls the subs ~5us, so
casts b1-b3 run on DVE). gt0/gt1 loads and colmaxes are chunked (6/5
keypoints) to start compute earlier; consts are packed into 2 tensors
on the scalar HWDGE ring; the last pred tile (3,1) is processed in
halves whose partial sums fold via an accumulating PSUM matmul.

Pitfalls measured on HW (do NOT revisit without remeasuring):
 - SWDGE cast-DMA shipping preds as fp8 derates EVERY engine clock
   ~1.2x for the whole run (115us); the same derate appears
   spontaneously on ~1 in 6 runs.
 - Splitting streams across both HWDGE rings caps each at ~256 GB/s
   and makes the Tile scheduler hoist all colmaxes before the subs.
 - tensor_tensor_reduce crashes the device; use scalar_tensor_tensor
   with accum_out for fused square+sum on DVE.
 - fp8 TensorTensor runs 1x (6.0us); DVE f32 reduce is 1 elem/cycle
   for any AP shape; f32->bf16 copy and bf16 TT run 2/cycle; ACT is
   1/cycle at 1.2GHz for all dtypes.

Argmax scheme (exact, first-occurrence tie-break like jnp.argmax):
 - colmax[p, j] = max_f gt[j][p, f]; PE-transpose -> [44,128]; global
   max -> winning partition p* (lowest tied p via max of mask*(128-p))
 - indirect-DMA gather of row (p*, j) for all 44 images -> [44, 512]
 - rowscore[j] = max_f (row >= gmax)*(512-f) = 512 - f* (max-reduce:
   exact first occurrence even if the winning value repeats)
 - flat = p**512 + f*; per-(image,stack) indirect-DMA gathers the C=7
   label-pred channels at the argmax locations ([B*S*HW, C] layout).
"""
import sys
import numpy as np

sys.path.insert(0, "/opt/trn_rl_repo")

import ml_dtypes
import concourse.bacc as bacc
import concourse.mybir as mybir
import concourse.tile as tile
from concourse.bass import IndirectOffsetOnAxis
from concourse.bass_utils import run_bass_kernel_spmd

F32 = mybir.dt.float32
BF16 = mybir.dt.bfloat16
I32 = mybir.dt.int32
F8 = mybir.dt.float8e4

HM_NP = ml_dtypes.bfloat16

B_LOC = 4      # batch per core
S = 2          # stacks
K = 11         # keypoints
C = 7          # label channels
HW = 65536     # 256*256
P = 128        # partitions
FK = HW // P   # 512
NJ = B_LOC * K  # 44 (b,k) images per core
NSC = S * C     # 14 (s,c) pairs
KF = K * FK     # 5632 free elems per (b[,s]) tile

# --- tuning knobs ---
# Casts alternate ACT/DVE per b so each engine's per-sample work (~12.5us)
# matches the 12.4us/b DMA pace: DVE = colmax 5.9 + 2 subs 5.9 (+cast 2.9 on
# its turn); ACT = 2 squares 10 (+cast 5 on its turn). Squares are ACT-only
# (tensor_tensor_reduce crashes the device; gpsimd can't run TT/reduce-X).
CAST_ACT = [True, False, False, False]  # gt cast engine per b (ACT else DVE)

_CACHE = {}


def _consts():
    # c44 packs all [NJ, *] constants: negp 128 | negf 512 | sbase 2 |
    # rowbase 1 | blockind 4 | labels placeholder 14 (labels are per-run,
    # shipped separately)
    negp = np.broadcast_to((P - np.arange(P, dtype=np.float32))[None, :], (NJ, P))
    negf = np.broadcast_to((FK - np.arange(FK, dtype=np.float32))[None, :], (NJ, FK))
    b_of_j = np.arange(NJ) // K
    k_of_j = np.arange(NJ) % K
    sbase = ((b_of_j[:, None] * S + np.arange(S)[None, :]) * HW).astype(np.float32)
    rowbase = ((b_of_j * P + P) * K + k_of_j).astype(np.float32)[:, None]
    blockind = (b_of_j[:, None] == np.arange(B_LOC)[None, :]).astype(np.float32)
    c44 = np.concatenate([negp, negf, sbase, rowbase, blockind], axis=1)
    # c128 packs identity 128 | ones 1
    c128 = np.concatenate([np.eye(P, dtype=np.float32),
                           np.ones((P, 1), np.float32)], axis=1)
    return np.ascontiguousarray(c44), np.ascontiguousarray(c128)


def _build():
    nc = bacc.Bacc("TRN2", target_bir_lowering=False, debug=False,
                   enable_asserts=False, num_devices=8)
    hm = nc.dram_tensor("hm", [B_LOC, S, P, KF], BF16, kind="ExternalInput").ap()
    gt = nc.dram_tensor("gt", [B_LOC * P * K, FK], F32, kind="ExternalInput").ap()
    lb = nc.dram_tensor("lb", [B_LOC * S * HW, C], F32, kind="ExternalInput").ap()
    labels_bc = nc.dram_tensor("labels_bc", [NJ, NSC], F32, kind="ExternalInput").ap()
    c44_d = nc.dram_tensor("c44", [NJ, 647], F32, kind="ExternalInput").ap()
    c128_d = nc.dram_tensor("c128", [P, P + 1], F32, kind="ExternalInput").ap()
    hm_out = nc.dram_tensor("hm_out", [1, B_LOC * S], F32, kind="ExternalOutput").ap()
    lb_out = nc.dram_tensor("lb_out", [B_LOC, S], F32, kind="ExternalOutput").ap()

    # direct-load view of gt: [b, p, (k f)] with contiguous partition rows
    gt3v = gt.rearrange("(b p k) f -> b p (k f)", b=B_LOC, p=P)

    with tile.TileContext(nc) as tc:
        with (
            tc.tile_pool(name="gtp", bufs=2) as gtp,
            tc.tile_pool(name="gbf", bufs=2) as gbfp,
            tc.tile_pool(name="work", bufs=3) as work,
            tc.tile_pool(name="diffp", bufs=3) as diffp,
            tc.tile_pool(name="small", bufs=1) as small,
            tc.tile_pool(name="psum", bufs=2, space="PSUM") as psp,
        ):
            c44_t = small.tile([NJ, 647], F32, tag="c44")
            c128_t = small.tile([P, P + 1], F32, tag="c128")
            lab_t = small.tile([NJ, NSC], F32, tag="lab")
            negp_t = c44_t[:, 0:P]
            negf_t = c44_t[:, P:P + FK]
            sbase_t = c44_t[:, P + FK:P + FK + S]
            rowb_t = c44_t[:, P + FK + S:P + FK + S + 1]
            blk_t = c44_t[:, P + FK + S + 1:P + FK + S + 1 + B_LOC]
            id_t = c128_t[:, 0:P]
            ones_t = c128_t[:, P:P + 1]

            colmax = small.tile([P, NJ], F32, tag="colmax")
            acc = small.tile([P, B_LOC * S], F32, tag="acc")
            scr_s = small.tile([P, KF], BF16, tag="scr_s")
            scr_v = small.tile([P, KF], BF16, tag="scr_v")

            gt_ts = {}
            gbf_ts = {}

            # consts ride the scalar (ACT) HWDGE ring; the big streams all
            # share the sync ring in consumption order (dual-ring splitting
            # capped each ring at ~256GB/s and made the Tile scheduler
            # reorder the DVE queue - measured 128us vs 104us this way).
            nc.scalar.dma_start(out=c44_t[:], in_=c44_d)
            nc.scalar.dma_start(out=c128_t[:], in_=c128_d)
            nc.scalar.dma_start(out=lab_t[:], in_=labels_bc)

            KSPL = 6            # gt0 chunk boundary (keypoints)
            FSPL = KSPL * FK    # 3072

            def load_gt(b):
                gt_t = gtp.tile([P, KF], F32, tag="gt")
                if b <= 1:
                    # chunked so the colmax can start on the first half
                    # while the second is still in flight
                    nc.sync.dma_start(out=gt_t[:, 0:FSPL],
                                      in_=gt3v[b][:, 0:FSPL])
                    nc.sync.dma_start(out=gt_t[:, FSPL:KF],
                                      in_=gt3v[b][:, FSPL:KF])
                else:
                    nc.sync.dma_start(out=gt_t[:], in_=gt3v[b])
                gt_ts[b] = gt_t

            def load_pred(b, s, split=False):
                # plain HWDGE bf16: SWDGE cast-DMA shipping (fp8->bf16)
                # derates every engine clock ~1.2x for the whole run.
                pred_t = work.tile([P, KF], BF16, tag="pred")
                if split:
                    nc.sync.dma_start(out=pred_t[:, 0:KF // 2],
                                      in_=hm[b, s, :, 0:KF // 2])
                    nc.sync.dma_start(out=pred_t[:, KF // 2:KF],
                                      in_=hm[b, s, :, KF // 2:KF])
                else:
                    nc.sync.dma_start(out=pred_t[:], in_=hm[b, s])
                return pred_t

            def proc_gt(b):
                gt_t = gt_ts[b]
                gt3 = gt_t[:].rearrange("p (k f) -> p k f", k=K)
                gbf_t = gbfp.tile([P, KF], BF16, tag="gbf")
                if b == 0:
                    # chunked: compute starts as soon as the first 6
                    # keypoints land instead of the whole 2.88MB tile
                    nc.vector.tensor_reduce(
                        out=colmax[:, 0:KSPL], in_=gt3[:, 0:KSPL],
                        axis=mybir.AxisListType.X, op=mybir.AluOpType.max)
                    nc.scalar.copy(out=gbf_t[:, 0:FSPL], in_=gt_t[:, 0:FSPL])
                    nc.vector.tensor_reduce(
                        out=colmax[:, KSPL:K], in_=gt3[:, KSPL:K],
                        axis=mybir.AxisListType.X, op=mybir.AluOpType.max)
                    nc.scalar.copy(out=gbf_t[:, FSPL:KF], in_=gt_t[:, FSPL:KF])
                elif b == 1:
                    nc.vector.tensor_reduce(
                        out=colmax[:, K:K + KSPL], in_=gt3[:, 0:KSPL],
                        axis=mybir.AxisListType.X, op=mybir.AluOpType.max)
                    nc.vector.tensor_reduce(
                        out=colmax[:, K + KSPL:2 * K], in_=gt3[:, KSPL:K],
                        axis=mybir.AxisListType.X, op=mybir.AluOpType.max)
                    nc.vector.tensor_copy(out=gbf_t[:], in_=gt_t[:])
                else:
                    nc.vector.tensor_reduce(
                        out=colmax[:, b * K:(b + 1) * K], in_=gt3,
                        axis=mybir.AxisListType.X, op=mybir.AluOpType.max)
                    if CAST_ACT[b]:
                        nc.scalar.copy(out=gbf_t[:], in_=gt_t[:])
                    else:
                        nc.vector.tensor_copy(out=gbf_t[:], in_=gt_t[:])
                gbf_ts[b] = gbf_t

            def proc_pred(b, s, pred_t, sq_dve=False):
                diff_t = diffp.tile([P, KF], BF16, tag="diff")
                nc.vector.tensor_tensor(
                    out=diff_t[:], in0=pred_t[:], in1=gbf_ts[b][:],
                    op=mybir.AluOpType.subtract)
                col = b * S + s
                if sq_dve:
                    nc.vector.scalar_tensor_tensor(
                        out=scr_v[:], in0=diff_t[:], scalar=1.0,
                        in1=diff_t[:], op0=mybir.AluOpType.mult,
                        op1=mybir.AluOpType.mult,
                        accum_out=acc[:, col:col + 1])
                else:
                    nc.scalar.activation(
                        out=scr_s[:], in_=diff_t[:],
                        func=mybir.ActivationFunctionType.Square,
                        accum_out=acc[:, col:col + 1])

            def proc_pred_half(b, s, pred_t, half, accum):
                # sub on DVE, then square+accum ALSO on DVE via the stock
                # STT op (out=(d*1)*d, accum=sum): the stream-tail squares
                # would otherwise serialize on ACT behind the earlier ones.
                lo, hi = (0, KF // 2) if half == 0 else (KF // 2, KF)
                diff_t = diffp.tile([P, KF // 2], BF16, tag="diffh")
                nc.vector.tensor_tensor(
                    out=diff_t[:], in0=pred_t[:, lo:hi],
                    in1=gbf_ts[b][:, lo:hi], op=mybir.AluOpType.subtract)
                nc.scalar.activation(
                    out=scr_s[:, 0:KF // 2], in_=diff_t[:],
                    func=mybir.ActivationFunctionType.Square,
                    accum_out=accum)

            # ---- interleaved pipeline (v2b emission pattern) ----
            load_gt(0)
            p00 = load_pred(0, 0)
            p01 = load_pred(0, 1)
            load_gt(1)
            p10 = load_pred(1, 0)
            p11 = load_pred(1, 1)
            proc_gt(0)
            proc_pred(0, 0, p00)
            proc_pred(0, 1, p01)
            # gt3 rides the ring right behind gt2: on slow-DMA runs its
            # colmax/cast then run mid-stream instead of serially after the
            # last byte (saves ~9us at 342GB/s; neutral at 409GB/s)
            load_gt(2)
            load_gt(3)
            p20 = load_pred(2, 0)
            p21 = load_pred(2, 1)
            proc_gt(1)
            proc_pred(1, 0, p10)
            proc_pred(1, 1, p11)
            p30 = load_pred(3, 0)
            p31 = load_pred(3, 1, split=True)
            proc_gt(2)
            proc_pred(2, 0, p20)
            proc_pred(2, 1, p21)
            proc_gt(3)

            # ---- argmax stage (small ops; hides under the stream tail) ----
            cm_p = psp.tile([NJ, P], F32, tag="cmp", space="PSUM")
            nc.tensor.transpose(out=cm_p[:], in_=colmax[:], identity=id_t)
            cmT = small.tile([NJ, P], F32, tag="cmT")
            nc.vector.tensor_copy(out=cmT[:], in_=cm_p[:])
            gmax = small.tile([NJ, 1], F32, tag="gmax")
            nc.vector.tensor_reduce(out=gmax[:], in_=cmT[:],
                                    axis=mybir.AxisListType.X,
                                    op=mybir.AluOpType.max)
            maskT = small.tile([NJ, P], F32, tag="maskT")
            nc.vector.tensor_scalar(out=maskT[:], in0=cmT[:], scalar1=gmax[:],
                                    scalar2=None, op0=mybir.AluOpType.is_ge)
            scoreT = small.tile([NJ, P], F32, tag="scoreT")
            nc.vector.tensor_tensor(out=scoreT[:], in0=maskT[:], in1=negp_t,
                                    op=mybir.AluOpType.mult)
            pscore = small.tile([NJ, 1], F32, tag="pscore")
            nc.vector.tensor_reduce(out=pscore[:], in_=scoreT[:],
                                    axis=mybir.AxisListType.X,
                                    op=mybir.AluOpType.max)
            # row units: off_row = rowbase - pscore*K
            offr_f = small.tile([NJ, 1], F32, tag="offr_f")
            nc.vector.scalar_tensor_tensor(
                out=offr_f[:], in0=pscore[:], scalar=float(-K),
                in1=rowb_t, op0=mybir.AluOpType.mult,
                op1=mybir.AluOpType.add)
            offr_i = small.tile([NJ, 1], I32, tag="offr_i")
            nc.vector.tensor_copy(out=offr_i[:], in_=offr_f[:])
            rows = small.tile([NJ, FK], F32, tag="rows")
            nc.gpsimd.indirect_dma_start(
                out=rows[:], out_offset=None, in_=gt,
                in_offset=IndirectOffsetOnAxis(ap=offr_i[:], axis=0))

            # ---- b3 pred work, interleaved with the argmax chain; the
            # last (3,1) tile is processed in halves so the post-stream
            # serial tail is half a tile, with the two partial sums folded
            # by an accumulating PSUM matmul (PE is idle anyway) ----
            proc_pred(3, 0, p30)
            accA = small.tile([P, 1], F32, tag="accA")
            accB = small.tile([P, 1], F32, tag="accB")
            proc_pred_half(3, 1, p31, 0, accA[:])
            proc_pred_half(3, 1, p31, 1, accB[:])

            # rowscore = max((row >= gmax) * (512 - f)) = 512 - f*  (max, not
            # sum: exact first-occurrence even if the winning value repeats)
            rmsk = small.tile([NJ, FK], F32, tag="rmsk")
            nc.vector.scalar_tensor_tensor(
                out=rmsk[:], in0=rows[:], scalar=gmax[:],
                in1=negf_t, op0=mybir.AluOpType.is_ge,
                op1=mybir.AluOpType.mult)
            rs44 = small.tile([NJ, 1], F32, tag="rs44")
            nc.vector.tensor_reduce(out=rs44[:], in_=rmsk[:],
                                    axis=mybir.AxisListType.X,
                                    op=mybir.AluOpType.max)
            # flat = (128-pscore)*512 + (512-rs44)
            t1 = small.tile([NJ, 1], F32, tag="t1")
            nc.vector.tensor_scalar(out=t1[:], in0=pscore[:], scalar1=-512.0,
                                    scalar2=None, op0=mybir.AluOpType.mult)
            flatf = small.tile([NJ, 1], F32, tag="flatf")
            nc.vector.scalar_tensor_tensor(
                out=flatf[:], in0=t1[:], scalar=float(P * FK + FK),
                in1=rs44[:], op0=mybir.AluOpType.add,
                op1=mybir.AluOpType.subtract)
            # lb is host-transposed to [B*S*HW, C]: one offset per
            # (image, stack) fetches all C=7 channels contiguously.
            off2_f = small.tile([NJ, S], F32, tag="off2_f")
            nc.vector.tensor_scalar(out=off2_f[:], in0=sbase_t,
                                    scalar1=flatf[:], scalar2=None,
                                    op0=mybir.AluOpType.add)
            off2_i = small.tile([NJ, S], I32, tag="off2_i")
            nc.vector.tensor_copy(out=off2_i[:], in_=off2_f[:])
            gath = small.tile([NJ, NSC], F32, tag="gath")
            for s in range(S):
                nc.gpsimd.indirect_dma_start(
                    out=gath[:, s * C:(s + 1) * C], out_offset=None,
                    in_=lb,
                    in_offset=IndirectOffsetOnAxis(ap=off2_i[:, s:s + 1], axis=0))

            # ---- label-loss tail + outputs ----
            ldiff = small.tile([NJ, NSC], F32, tag="ldiff")
            nc.vector.tensor_tensor(out=ldiff[:], in0=gath[:], in1=lab_t[:],
                                    op=mybir.AluOpType.subtract)
            lsq = small.tile([NJ, NSC], F32, tag="lsq")
            nc.scalar.activation(out=lsq[:], in_=ldiff[:],
                                 func=mybir.ActivationFunctionType.Square)
            persum = small.tile([NJ, S], F32, tag="persum")
            nc.vector.tensor_reduce(
                out=persum[:],
                in_=lsq[:].rearrange("j (s c) -> j s c", s=S),
                axis=mybir.AxisListType.X, op=mybir.AluOpType.add)
            lb_p = psp.tile([B_LOC, S], F32, tag="lbp", space="PSUM")
            nc.tensor.matmul(out=lb_p[:], lhsT=blk_t, rhs=persum[:],
                             start=True, stop=True)
            lb_s = small.tile([B_LOC, S], F32, tag="lbs")
            nc.scalar.activation(out=lb_s[:], in_=lb_p[:],
                                 func=mybir.ActivationFunctionType.Copy,
                                 scale=1.0 / (K * C))
            nc.sync.dma_start(out=lb_out, in_=lb_s[:])

            # col 7's sum lives in accA/accB (the (3,1) tile was processed
            # in halves); fold them with an accumulating PSUM matmul.
            hm_p = psp.tile([1, B_LOC * S], F32, tag="hmp", space="PSUM")
            nc.tensor.matmul(out=hm_p[:, 0:7], lhsT=ones_t, rhs=acc[:, 0:7],
                             start=True, stop=True)
            nc.tensor.matmul(out=hm_p[:, 7:8], lhsT=ones_t, rhs=accA[:],
                             start=True, stop=False)
            nc.tensor.matmul(out=hm_p[:, 7:8], lhsT=ones_t, rhs=accB[:],
                             start=False, stop=True)
            hm_s = small.tile([1, B_LOC * S], F32, tag="hms")
            nc.scalar.activation(out=hm_s[:], in_=hm_p[:],
                                 func=mybir.ActivationFunctionType.Copy,
                                 scale=1.0 / (K * HW))
            nc.sync.dma_start(out=hm_out, in_=hm_s[:])

    nc.compile()
    return nc


def _get_nc():
    if "nc" not in _CACHE:
        _CACHE["nc"] = _build()
    return _CACHE["nc"]


def make_in_maps(combined_hm_preds, combined_lb_preds, heatmaps, labels):
    c44, c128 = _consts()
    in_maps = []
    for c in range(8):
        sl = slice(c * B_LOC, (c + 1) * B_LOC)
        lab = np.asarray(labels[sl], np.float32)  # [4, 11, 7]
        lab_bc = np.broadcast_to(
            lab[:, :, None, :], (B_LOC, K, S, C)).reshape(NJ, NSC)
        # pre-transpose so partition rows are contiguous DMA lines
        hm_r = np.asarray(combined_hm_preds[sl], np.float32).reshape(
            B_LOC, S, K, P, FK).transpose(0, 1, 3, 2, 4).reshape(
            B_LOC, S, P, KF).astype(HM_NP)
        gt_r = np.asarray(heatmaps[sl], np.float32).reshape(
            B_LOC, K, P, FK).transpose(0, 2, 1, 3).reshape(B_LOC * P * K, FK)
        lb_r = np.asarray(combined_lb_preds[sl], np.float32).reshape(
            B_LOC, S, C, HW).transpose(0, 1, 3, 2).reshape(B_LOC * S * HW, C)
        m = {
            "hm": np.ascontiguousarray(hm_r),
            "gt": np.ascontiguousarray(gt_r),
            "lb": np.ascontiguousarray(lb_r),
            "labels_bc": np.ascontiguousarray(lab_bc),
            "c44": c44,
            "c128": c128,
        }
        in_maps.append(m)
    return in_maps


def run(in_maps, trace=False, **kw):
    nc = _get_nc()
    return run_bass_kernel_spmd(nc, in_maps, list(range(8)), trace=trace, **kw)


def kernel(combined_hm_preds, combined_lb_preds, heatmaps, labels):
    in_maps = make_in_maps(combined_hm_preds, combined_lb_preds, heatmaps,
                           labels)
    res = run(in_maps).results
    combined = np.concatenate(
        [r["hm_out"].reshape(B_LOC, S) for r in res], axis=0)
    labels_loss = np.concatenate([r["lb_out"] for r in res], axis=0)
    return combined.astype(np.float32), labels_loss.astype(np.float32)
